# revision 1
# baseline (speedup 1.0000x reference)
"""Dense dot-product attention (B=1, H=16, S=4096, D=64, fp32) on 8 trn2 cores.

Head-parallel: core c computes heads [2c, 2c+1] fully on-device, no comms.

Per-head device algorithm (flash-style, S^T layout):
  S^T[k, q] = K @ Q^T      (contraction over d=64 on partitions; fp32r matmuls)
  P^T = exp(S^T - 40)      (ACT engine, PSUM -> SBUF; softmax is shift-invariant)
  outT[d', q] = V'^T @ P^T (V' = [V | 1] so row 64 accumulates the softmax sums)
  out[q, d'] = PE-transpose(outT chunk) ; out[:, :64] *= 1/out[:, 64]
               (per-partition scalar multiply; output stored natural [s, d])

Host pre-transposes Q/K into [h, d, s] when sharding (pure data marshalling).
"""

import sys

if "/opt/trn_rl_repo" not in sys.path:
    sys.path.insert(0, "/opt/trn_rl_repo")

import numpy as np

B, H, S, D = 1, 16, 4096, 64
N_CORES = 8
HEADS_PER_CORE = H // N_CORES  # 2

KT = S // 128        # 32 k-tiles per head
GROUP = 1024         # q columns per softmax staging group (2 PSUM banks)
NG = S // GROUP      # 4 groups per head
CHUNK = 512          # matmul moving-dim (one PSUM bank)
NCHUNK = GROUP // CHUNK  # 2
NJ = CHUNK // 128    # q-tiles per chunk for the output transpose
# shift chosen so the DVE exp path's magic rounding uses the integer 58:
# exp(x + EXP_BIAS) with EXP_BIAS = -58/log2(e); softmax is shift-invariant.
EXP_BIAS = -40.20261913005731
DVE_EVERY = 6     # 1-in-6 k-tiles take the DVE exp path

_compiled = None

# DVE exp path: x' = x*log2(e) (stock pre-scale), tau = x' - 58, then
# 2^tau = 2^m * 2^f with m = round(tau) via magic-number rounding and
# f in [-1/2, 1/2]:
#   op A: (max(m + 127, 0)) * 2^23 built arithmetically, written as int32;
#         the bitcast of that int32 is exactly 2^m (underflow clamps to 0).
#   op B: quadratic 1 + f*(B + A*f), max rel err 1.96e-3 on [-1/2, 1/2].
#   a stock multiply fuses them into P^T. Neither op uses Src1 (that read
#   path crashes this terminal's DVE firmware - even for production ops).
LOG2E = 1.4426950408889634
LN2 = 0.6931471805599453
MAGIC = 12582912.0               # 1.5 * 2^23: fp32 add rounds to integer
MAGIC_ADD = MAGIC - 58.0         # r = x' + MAGIC_ADD -> MAGIC + round(x'-58)
MAGIC_SUB = MAGIC - 127.0        # r - MAGIC_SUB = m + 127
QUAD_A = 0.23986402898180526
QUAD_B = 0.7029417939863177


def _register_dve_exp_ops():
    import concourse.dve_ops as dve_ops
    from concourse.dve_ops import DveOp, OPS, has_src1
    from concourse.dve_spec import Spec, Src0, Src1, C0, C1, C2, Zero, One, maxx, lower
    from concourse.dve_uop import DveOpSpec
    from concourse.dve_table_gen import dve_ver_for
    import numpy as np

    if "EXP2_INT_ANT" in dve_ops._SUB_OPCODE_FOR_NAME:
        by_name = {op.name: op for op in OPS}
        return by_name["EXP2_INT_ANT"], by_name["EXP2_FRAC_ANT"]

    f32 = np.float32

    def ref_a(in0, in1, s0, s1, imm2):
        x = in0.astype(np.float32)
        r = x + f32(s0)
        u = np.maximum(r - f32(s1), f32(0.0))
        return u * f32(imm2)

    def ref_b(in0, in1, s0, s1, imm2):
        x = in0.astype(np.float32)
        r = x + f32(s0)
        s = r - f32(s0)
        fr = x - s
        return (fr * f32(s1) + f32(imm2)) * fr + f32(1.0)

    _ra = Src0 + C0
    op_a = DveOp(
        "EXP2_INT_ANT",
        Spec(body=maxx(_ra - C1, Zero) * C2, reference=ref_a),
        subdim=False,
        uops_sha={},
    )
    _rb = Src0 + C0
    _fb = Src0 - (_rb - C0)
    op_b = DveOp(
        "EXP2_FRAC_ANT",
        Spec(body=(_fb * C1 + C2) * _fb + One, reference=ref_b),
        subdim=False,
        uops_sha={},
    )
    for op in (op_a, op_b):
        OPS.append(op)
        dve_ops.CUSTOM_DVE_SPECS[op.name] = op.spec
        dve_ops._SUB_OPCODE_FOR_NAME[op.name] = (
            dve_ops._CUSTOM_DVE_ROW_BASE + len(dve_ops._SUB_OPCODE_FOR_NAME))
        for ver in ("v3", "v4"):
            try:
                compiled = DveOpSpec(
                    name=op.name,
                    opcode=dve_ops._SUB_OPCODE_FOR_NAME[op.name],
                    uops=lower(op.spec, ver=ver),
                    rd1_en=has_src1(op.spec),
                )
                op.uops_sha[ver] = compiled.sha(ver)
            except Exception:
                pass
    return op_a, op_b


def _build():
    import concourse.bacc as bacc
    import concourse.mybir as mybir
    import concourse.tile as tile
    from concourse.masks import make_identity

    op_exp_int, op_exp_frac = _register_dve_exp_ops()

    f32 = mybir.dt.float32
    f32r = mybir.dt.float32r
    i32 = mybir.dt.int32

    nc = bacc.Bacc("TRN2", target_bir_lowering=False, debug=False,
                   num_devices=N_CORES)

    # qT/kT arrive duplicated across partition halves (rows 64:128 = rows
    # 0:64) so adjacent k-tiles' QK matmuls pack into disjoint PE row-groups
    # (tile_position (0,0) / (64,0)) and run concurrently.
    qT = nc.dram_tensor("qT", [HEADS_PER_CORE, 128, S], f32r, kind="ExternalInput")
    kT = nc.dram_tensor("kT", [HEADS_PER_CORE, 128, S], f32r, kind="ExternalInput")
    # v arrives with a ones column appended (so PV accumulates softmax sums)
    v = nc.dram_tensor("v", [HEADS_PER_CORE, S, D + 1], f32r, kind="ExternalInput")
    outT = nc.dram_tensor("outT", [HEADS_PER_CORE, D, S], f32, kind="ExternalOutput")
    # final group of the final head lands here already transposed ([q, d])
    out2 = nc.dram_tensor("out2", [GROUP, D], f32, kind="ExternalOutput")

    with tile.TileContext(nc) as tc:
        with (
            tc.tile_pool(name="qk", bufs=2) as qk_pool,
            tc.tile_pool(name="vp", bufs=2) as vp_pool,
            tc.tile_pool(name="pt", bufs=6) as pt_pool,
            tc.tile_pool(name="ou", bufs=4) as ou_pool,
            tc.tile_pool(name="ob", bufs=3) as ob_pool,
            tc.tile_pool(name="small", bufs=1) as small_pool,
            tc.tile_pool(name="rcp", bufs=4) as rcp_pool,
            tc.tile_pool(name="ei", bufs=3) as ei_pool,
            tc.tile_pool(name="dram", bufs=4, space="DRAM") as dram_pool,
            tc.tile_pool(name="psum_s", bufs=3, space="PSUM") as psum_s,
            tc.tile_pool(name="psum_o", bufs=2, space="PSUM") as psum_o,
        ):
            bias_t = small_pool.tile([128, 1], f32, tag="bias")
            nc.gpsimd.memset(bias_t, EXP_BIAS)
            ident = small_pool.tile([D + 1, D + 1], f32, tag="ident")
            make_identity(nc, ident)
            # dummy exp so the ACT table set loads during the input DMAs
            warm_t = small_pool.tile([128, 1], f32, tag="warm")
            nc.scalar.activation(out=warm_t, in_=bias_t,
                                 func=mybir.ActivationFunctionType.Exp,
                                 bias=bias_t[:], scale=1.0)

            for h in range(HEADS_PER_CORE):
                # --- per-head loads, split so the first group can start early ---
                qt_t = qk_pool.tile([128, S], f32r, tag="qt")
                kt_t = qk_pool.tile([128, S], f32r, tag="kt")
                vp_t = vp_pool.tile([128, KT, D + 1], f32r, tag="vp")
                if h == 0:
                    # tiny first slices so the first QK matmul starts ASAP;
                    # kt goes out on the ACT-side HWDGE queue (idle at t=0)
                    # so the two dispatches don't serialize.
                    nc.scalar.dma_start(out=kt_t[:, 0:256], in_=kT[h][:, 0:256])
                    nc.sync.dma_start(out=qt_t[:, 0:CHUNK], in_=qT[h][:, 0:CHUNK])
                    nc.sync.dma_start(out=qt_t[:, CHUNK:GROUP],
                                      in_=qT[h][:, CHUNK:GROUP])
                    nc.scalar.dma_start(out=kt_t[:, 256:GROUP],
                                        in_=kT[h][:, 256:GROUP])
                for g in range(NG):
                    sl = slice(g * GROUP, (g + 1) * GROUP)
                    if h == 0 and g == 0:
                        pass
                    else:
                        nc.sync.dma_start(out=kt_t[:, sl], in_=kT[h][:, sl])
                        nc.sync.dma_start(out=qt_t[:, sl], in_=qT[h][:, sl])
                    ksl = slice(g * (KT // NG), (g + 1) * (KT // NG))
                    nc.sync.dma_start(
                        out=vp_t[:, ksl, :],
                        in_=v[h].rearrange("(kt p) e -> p kt e", p=128)[:, ksl, :],
                    )

                for g in range(NG):
                    q0 = g * GROUP
                    pv_ps = [psum_o.tile([D + 1, CHUNK], f32, tag="pv",
                                         name=f"pv_{h}_{g}_{c}")
                             for c in range(NCHUNK)]
                    pv_started = [False] * NCHUNK

                    def emit_pv(kt_i, pt_t, last=False):
                        for c in range(NCHUNK):
                            nc.tensor.matmul(
                                pv_ps[c],
                                lhsT=vp_t[:, kt_i, :],
                                rhs=pt_t[:, c * CHUNK:(c + 1) * CHUNK],
                                start=(not pv_started[c]), stop=last,
                                skip_group_check=True,
                            )
                            pv_started[c] = True

                    deferred = []  # (release_at, kt_i, pt_t) for DVE-exp tiles
                    st_pair = {}
                    for kt_i in range(KT):
                        if kt_i % 2 == 0:
                            # emit the QK matmuls for this k-tile PAIR: even
                            # tile on PE rows 0:64, odd tile on rows 64:128 -
                            # disjoint row-groups execute concurrently.
                            for j in (0, 1):
                                st_pair[kt_i + j] = psum_s.tile(
                                    [128, GROUP], f32, tag="st",
                                    name=f"st_{h}_{g}_{kt_i + j}")
                            for c in range(NCHUNK):
                                for j in (0, 1):
                                    kk = kt_i + j
                                    rows = slice(64 * j, 64 * (j + 1))
                                    nc.tensor.matmul(
                                        st_pair[kk][:, c * CHUNK:(c + 1) * CHUNK],
                                        lhsT=kt_t[rows, kk * 128:(kk + 1) * 128],
                                        rhs=qt_t[rows,
                                                 q0 + c * CHUNK:q0 + (c + 1) * CHUNK],
                                        start=True, stop=True,
                                    )
                        st_ps = st_pair.pop(kt_i)
                        # exp(S^T - 40): most k-tiles on ACT; every DVE_EVERYth
                        # on the vector engine via the custom 2^m * quad(2^f)
                        # ops. DVE-tile PV matmuls are deferred 2 k-tiles so
                        # the longer DVE latency never blocks the PE queue
                        # (PSUM accumulation order is irrelevant).
                        pt_t = pt_pool.tile([128, GROUP], f32r, tag="pt")
                        if kt_i % DVE_EVERY == 3:
                            sx_t = ei_pool.tile([128, GROUP], f32, tag="sx")
                            nc.vector.tensor_copy(sx_t, st_ps)
                            e_t = ei_pool.tile([128, GROUP], i32, tag="ei")
                            p_t = ei_pool.tile([128, GROUP], f32, tag="pq")
                            nc.vector._custom_dve(
                                op_exp_int, out=e_t, in0=sx_t,
                                s0=MAGIC_ADD, s1=MAGIC_SUB, imm2=8388608.0)
                            nc.vector._custom_dve(
                                op_exp_frac, out=p_t, in0=sx_t,
                                s0=MAGIC_ADD, s1=QUAD_A, imm2=QUAD_B)
                            nc.vector.tensor_mul(pt_t, e_t.bitcast(f32), p_t)
                            deferred.append((min(kt_i + 6, KT - 2), kt_i, pt_t))
                        else:
                            nc.scalar.activation(
                                out=pt_t, in_=st_ps,
                                func=mybir.ActivationFunctionType.Exp,
                                bias=bias_t[:], scale=LN2,
                            )
                            emit_pv(kt_i, pt_t,
                                    last=(kt_i == KT - 1 and not deferred))
                        while deferred and deferred[0][0] <= kt_i:
                            _, dkt, dpt = deferred.pop(0)
                            emit_pv(dkt, dpt)
                    for di, (_, dkt, dpt) in enumerate(deferred):
                        emit_pv(dkt, dpt, last=(di == len(deferred) - 1))

                    q0 = g * GROUP
                    if h == HEADS_PER_CORE - 1 and g == NG - 1:
                        # --- final group: PE-transpose normalize (short tail;
                        # nothing follows, so borrowing psum is harmless) ---
                        for c in range(NCHUNK):
                            ou_t = ou_pool.tile([D + 1, CHUNK], f32, tag="ouf",
                                                name=f"ouf_{c}")
                            nc.vector.tensor_copy(ou_t, pv_ps[c])
                            ob_t = ob_pool.tile([128, NJ, D], f32, tag="ob",
                                                name=f"ob_{c}")
                            for j in range(NJ):
                                # per-j PSUM tiles (distinct banks) so the
                                # transposes don't serialize against the
                                # reciprocal/multiply readers
                                tr_ps = psum_s.tile([128, D + 1], f32, tag="st",
                                                    name=f"tr_{c}_{j}")
                                nc.tensor.transpose(
                                    tr_ps,
                                    ou_t[:, j * 128:(j + 1) * 128],
                                    ident,
                                )
                                rcp_t = rcp_pool.tile([128, 1], f32, tag="rcpf",
                                                      name=f"rcpf_{c}_{j}")
                                nc.vector.reciprocal(out=rcp_t,
                                                     in_=tr_ps[:, D:D + 1])
                                nc.vector.tensor_scalar_mul(
                                    ob_t[:, j, :], tr_ps[:, 0:D], rcp_t)
                            nc.sync.dma_start(
                                out=out2.rearrange("(j p) d -> p j d", p=128)[
                                    :, c * NJ:(c + 1) * NJ, :],
                                in_=ob_t,
                            )
                        continue

                    # --- per-group normalize via DRAM-bounced reciprocal ---
                    ou_t = ou_pool.tile([D + 1, GROUP], f32, tag="ou",
                                        name=f"ou_{h}_{g}")
                    for c in range(NCHUNK):
                        nc.vector.tensor_copy(
                            ou_t[:, c * CHUNK:(c + 1) * CHUNK], pv_ps[c])
                    sums_d = dram_pool.tile([GROUP], f32, tag="sums",
                                            name=f"sums_{h}_{g}")
                    nc.sync.dma_start(out=sums_d.rearrange("(o s) -> o s", o=1),
                                      in_=ou_t[D:D + 1, :])
                    sums_t = rcp_pool.tile([128, GROUP // 128], f32, tag="sums_t",
                                           name=f"sums_t_{h}_{g}")
                    nc.sync.dma_start(out=sums_t,
                                      in_=sums_d.rearrange("(p j) -> p j", p=128))
                    rcp_t = rcp_pool.tile([128, GROUP // 128], f32, tag="rcp_t",
                                          name=f"rcp_t_{h}_{g}")
                    nc.vector.reciprocal(out=rcp_t, in_=sums_t)
                    rcp_d = dram_pool.tile([GROUP], f32, tag="rcp",
                                           name=f"rcp_{h}_{g}")
                    nc.sync.dma_start(out=rcp_d.rearrange("(p j) -> p j", p=128),
                                      in_=rcp_t)
                    rep_t = ob_pool.tile([D, GROUP], f32, tag="rep",
                                         name=f"rep_{h}_{g}")
                    nc.sync.dma_start(
                        out=rep_t,
                        in_=rcp_d.rearrange("(o s) -> o s", o=1).to_broadcast((D, GROUP)),
                    )
                    nc.vector.tensor_mul(ou_t[0:D, :], ou_t[0:D, :], rep_t)
                    nc.sync.dma_start(out=outT[h][:, q0:q0 + GROUP],
                                      in_=ou_t[0:D, :])

    nc.compile()
    return nc


def _get_compiled():
    global _compiled
    if _compiled is None:
        _compiled = _build()
    return _compiled


def kernel(query: np.ndarray, key: np.ndarray, value: np.ndarray) -> np.ndarray:
    from concourse.bass_utils import run_bass_kernel_spmd

    nc = _get_compiled()

    q = np.asarray(query, dtype=np.float32).reshape(H, S, D)
    k = np.asarray(key, dtype=np.float32).reshape(H, S, D)
    v = np.asarray(value, dtype=np.float32).reshape(H, S, D)

    in_maps = []
    for c in range(N_CORES):
        hs = slice(c * HEADS_PER_CORE, (c + 1) * HEADS_PER_CORE)
        in_maps.append({
            "qT": np.ascontiguousarray(
                np.concatenate([q[hs].transpose(0, 2, 1)] * 2, axis=1)),
            # K pre-scaled by log2(e): the QK matmul then yields x*log2e,
            # which the DVE exp path consumes directly (ACT re-scales by ln2)
            "kT": np.ascontiguousarray(
                np.concatenate([k[hs].transpose(0, 2, 1)] * 2, axis=1))
                * np.float32(LOG2E),
            "v": np.concatenate(
                [v[hs], np.ones((HEADS_PER_CORE, S, 1), np.float32)], axis=-1),
        })

    res = run_bass_kernel_spmd(nc, in_maps, list(range(N_CORES)))

    out = np.empty((B, H, S, D), dtype=np.float32)
    for c in range(N_CORES):
        for hh in range(HEADS_PER_CORE):
            out[0, c * HEADS_PER_CORE + hh] = res.results[c]["outT"][hh].T
        out[0, c * HEADS_PER_CORE + HEADS_PER_CORE - 1, S - GROUP:] = \
            res.results[c]["out2"]
    return out



# revision 7
# speedup vs baseline: 1.0718x; 1.0718x over previous
"""Dense dot-product attention (B=1, H=16, S=4096, D=64, fp32) on 8 trn2 cores.

Head-parallel: core c computes heads [2c, 2c+1] fully on-device, no comms.

Per-head algorithm (S^T score layout, [q,d] output layout):
  scores: S^T[k, q] PSUM tile per (k-tile, q-group) via f32r matmuls with a
    65-row contraction: rows 0:64 contract d (K pre-scaled by 128*log2e),
    row 64 is a bias row adding SROW = 128*(127 - SH) - 64, so the PSUM
    value is v = 128*(x + 127) - 64 with x = score*log2e - SH.
  exp -> P (bf16) via two engine paths, split per k-tile:
    ACT path: P = Exp(v * ln2/128 + bias), full precision, bf16 out.
    DVE+Pool path ("expbits"): DVE custom op computes
        w = (C1*_t + C2)*_t + v,  _t = v - 128*round(v/128)  (magic round)
      then Pool adds K and converts to int16; the int16 IS the bf16 bit
      pattern of 2^x (exponent from the 128-quantized part, mantissa from
      the quadratic 2^frac fit; max weight err ~1%, rel out err ~3e-3).
  PV: out[q, d'] accumulates over 32 k-tiles via bf16 matmuls with
    lhsT = P^T[k, q-subtile] (ap=65/matmul), rhs = V'[k, 0:65] where V' has
    a ones column so out[q, 64] = sum of P (softmax denominator).
  normalize: per q-subtile reciprocal + per-partition scalar multiply;
    output written in natural [q, d] layout (no transposes).

PV matmuls are emitted LAG k-tiles behind QK so the exp chain latency never
stalls the in-order PE queue.
"""

import sys

if "/opt/trn_rl_repo" not in sys.path:
    sys.path.insert(0, "/opt/trn_rl_repo")

import numpy as np

B, H, S, D = 1, 16, 4096, 64
N_CORES = 8
HPC = H // N_CORES    # heads per core = 2

KT = S // 128         # 32 k-tiles per head
GROUP = 1024          # q columns per score group
NG = S // GROUP       # 4 groups per head
NJ = GROUP // 128     # 8 q-subtiles per group

LOG2E = 1.4426950408889634
LN2 = 0.6931471805599453
SH = 46.0                                   # shift in log2 units
SROW = 128.0 * (127.0 - SH) - 64.0          # bias-row constant (J = -64)
ACT_SCALE = LN2 / 128.0
ACT_BIAS = -(127.0 - 64.0 / 128.0) * LN2    # undoes 128*( . +127) - 64

# expbits constants: v + quad(_t) + K == bf16 bits of 2^x (see numcheck)
C0_MAGIC = 1.5 * 2**30
C1_QUAD = 2.459070897941e-03
C2_QUAD = -1.995185412854e-02
K_BITS = 53.044930589134

# k-tiles handled by the DVE+Pool expbits path (rest go to ACT).
# Keep the last tiles of each group on ACT (shorter latency tail).
DVE_SET = frozenset(int(round(x)) for x in np.linspace(0, 26, 13))
LAG_ACT = 3
LAG_DVE = 6
DEBUG_DUMP = False

_compiled = None


def _register_expbits_op():
    import concourse.dve_ops as dve_ops
    from concourse.dve_ops import DveOp, OPS, has_src1
    from concourse.dve_spec import Spec, Src0, C0, C1, C2, lower
    from concourse.dve_uop import DveOpSpec

    if "EXPBITS_ANT" in dve_ops._SUB_OPCODE_FOR_NAME:
        return {op.name: op for op in OPS}["EXPBITS_ANT"]

    from concourse.dve_spec import Zero, maxx

    f32 = np.float32

    def ref(in0, in1, s0, s1, imm2):
        x = in0.astype(np.float32)
        r = x + f32(s0)
        s = r - f32(s0)
        t = x - s
        return np.maximum((t * f32(s1) + f32(imm2)) * t + x, f32(0.0))

    _r = Src0 + C0
    _t = Src0 - (_r - C0)
    op = DveOp(
        "EXPBITS_ANT",
        # max(.., 0): deeply negative scores (x < -127) would otherwise go
        # negative in int16 and bitcast to huge negative bf16 weights.
        Spec(body=maxx((_t * C1 + C2) * _t + Src0, Zero), reference=ref),
        subdim=False,
        uops_sha={},
    )
    OPS.append(op)
    dve_ops.CUSTOM_DVE_SPECS[op.name] = op.spec
    dve_ops._SUB_OPCODE_FOR_NAME[op.name] = (
        dve_ops._CUSTOM_DVE_ROW_BASE + len(dve_ops._SUB_OPCODE_FOR_NAME))
    for ver in ("v3", "v4"):
        try:
            compiled = DveOpSpec(
                name=op.name,
                opcode=dve_ops._SUB_OPCODE_FOR_NAME[op.name],
                uops=lower(op.spec, ver=ver),
                rd1_en=has_src1(op.spec),
            )
            op.uops_sha[ver] = compiled.sha(ver)
        except Exception:
            pass
    return op


def _build():
    import concourse.bacc as bacc
    import concourse.mybir as mybir
    import concourse.tile as tile

    op_expbits = _register_expbits_op()

    f32 = mybir.dt.float32
    f32r = mybir.dt.float32r
    bf16 = mybir.dt.bfloat16
    i16 = mybir.dt.int16

    nc = bacc.Bacc("TRN2", target_bir_lowering=False, debug=False,
                   num_devices=N_CORES)

    qT = nc.dram_tensor("qT", [HPC, D + 1, S], f32r, kind="ExternalInput")
    kT = nc.dram_tensor("kT", [HPC, D + 1, S], f32r, kind="ExternalInput")
    v = nc.dram_tensor("v", [HPC, S, D + 1], bf16, kind="ExternalInput")
    out = nc.dram_tensor("out", [HPC, S, D], f32, kind="ExternalOutput")
    dbg = None
    if DEBUG_DUMP:
        dbg = nc.dram_tensor("dbg", [HPC, NG, 128, NJ, 128], f32,
                             kind="ExternalOutput")

    with tile.TileContext(nc) as tc:
        with (
            tc.tile_pool(name="qk", bufs=2) as qk_pool,
            tc.tile_pool(name="vp", bufs=2) as vp_pool,
            tc.tile_pool(name="ptb", bufs=5) as ptb_pool,
            tc.tile_pool(name="pti", bufs=8) as pti_pool,
            tc.tile_pool(name="wk", bufs=8) as wk_pool,
            tc.tile_pool(name="osb", bufs=2) as osb_pool,
            tc.tile_pool(name="rcp", bufs=16) as rcp_pool,
            tc.tile_pool(name="small", bufs=1) as small_pool,
            tc.tile_pool(name="psum_st", bufs=3, space="PSUM") as psum_st,
            tc.tile_pool(name="psum_oa", bufs=1, space="PSUM") as psum_oa,
            tc.tile_pool(name="psum_ob", bufs=1, space="PSUM") as psum_ob,
        ):
            bias_t = small_pool.tile([128, 1], f32, tag="bias")
            nc.gpsimd.memset(bias_t, ACT_BIAS)
            # dummy exp so the ACT table set loads during the input DMAs
            warm_t = small_pool.tile([128, 1], f32, tag="warm")
            nc.scalar.activation(out=warm_t, in_=bias_t,
                                 func=mybir.ActivationFunctionType.Exp,
                                 bias=bias_t[:], scale=1.0)

            # ---- input loads (h loop below re-loads per head; first head
            # split so the first QK can start early) ----
            kt_ts, qt_ts, vp_ts = {}, {}, {}
            for h in range(HPC):
                kt_ts[h] = qk_pool.tile([D + 1, S], f32r, tag="kt",
                                        name=f"kt_{h}")
                qt_ts[h] = qk_pool.tile([D + 1, S], f32r, tag="qt",
                                        name=f"qt_{h}")
                vp_ts[h] = vp_pool.tile([128, KT, D + 1], bf16, tag="vp",
                                        name=f"vp_{h}")
            # h0: full kT first (QK needs all k columns), then first q group
            nc.sync.dma_start(out=kt_ts[0], in_=kT[0])
            nc.sync.dma_start(out=qt_ts[0][:, 0:GROUP], in_=qT[0][:, 0:GROUP])
            nc.scalar.dma_start(
                out=vp_ts[0],
                in_=v[0].rearrange("(kt p) e -> p kt e", p=128))
            nc.sync.dma_start(out=qt_ts[0][:, GROUP:S], in_=qT[0][:, GROUP:S])
            nc.sync.dma_start(out=kt_ts[1], in_=kT[1])
            nc.sync.dma_start(out=qt_ts[1], in_=qT[1])
            nc.scalar.dma_start(
                out=vp_ts[1],
                in_=v[1].rearrange("(kt p) e -> p kt e", p=128))

            # ---- flat pipeline over (h, g, kt) ----
            steps = [(h, g, kt)
                     for h in range(HPC) for g in range(NG) for kt in range(KT)]
            pending = []          # (release_at_step, h, g, kt, pt_bf16_ap)
            group_state = {}      # (h, g) -> dict(oa, ob, started, flushed)

            def get_group(h, g):
                key = (h, g)
                if key not in group_state:
                    group_state[key] = {
                        "oa": psum_oa.tile([128, 4, 128], f32, tag="oa",
                                           name=f"oa_{h}_{g}"),
                        "ob": psum_ob.tile([128, 4, 128], f32, tag="ob",
                                           name=f"ob_{h}_{g}"),
                        "bank_started": [False, False],
                        "flushed": 0,
                    }
                return group_state[key]

            def emit_pv(h, g, kt, pt_b):
                gs = get_group(h, g)
                last = (kt == KT - 1)
                for j in range(NJ):
                    bank = 0 if j < 4 else 1
                    o_ps = gs["oa"] if j < 4 else gs["ob"]
                    # start=True clears has_written for the WHOLE bank, so
                    # it must be issued exactly once per bank; later writes
                    # to virgin elements overwrite, the rest accumulate.
                    nc.tensor.matmul(
                        o_ps[:, j % 4, 0:D + 1],
                        lhsT=pt_b[:, j * 128:(j + 1) * 128],
                        rhs=vp_ts[h][:, kt, :],
                        start=(not gs["bank_started"][bank]), stop=last,
                        skip_group_check=True,
                    )
                    gs["bank_started"][bank] = True
                gs["flushed"] += 1
                if gs["flushed"] == KT:
                    emit_norm(h, g)

            def emit_norm(h, g):
                gs = group_state[(h, g)]
                if DEBUG_DUMP:
                    dbg_sb = osb_pool.tile([128, NJ, 128], f32, tag="dsb",
                                           name=f"dsb_{h}_{g}")
                    for j in range(NJ):
                        o_ps = gs["oa"] if j < 4 else gs["ob"]
                        nc.vector.tensor_copy(dbg_sb[:, j, :],
                                              o_ps[:, j % 4, :])
                    nc.sync.dma_start(out=dbg[h][g], in_=dbg_sb)
                out_sb = osb_pool.tile([128, NJ, D], f32, tag="osb",
                                       name=f"osb_{h}_{g}")
                for j in range(NJ):
                    o_ps = gs["oa"] if j < 4 else gs["ob"]
                    rcp_t = rcp_pool.tile([128, 1], f32, tag="rcp",
                                          name=f"rcp_{h}_{g}_{j}")
                    nc.vector.reciprocal(out=rcp_t,
                                         in_=o_ps[:, j % 4, D:D + 1])
                    nc.vector.tensor_scalar_mul(
                        out_sb[:, j, :], o_ps[:, j % 4, 0:D], rcp_t)
                nc.sync.dma_start(
                    out=out[h][g * GROUP:(g + 1) * GROUP, :].rearrange(
                        "(j p) d -> p j d", p=128),
                    in_=out_sb,
                )
                del group_state[(h, g)]

            for n, (h, g, kt) in enumerate(steps):
                q0 = g * GROUP
                # QK: 2 matmuls of 512 columns, 65-row contraction
                st_t = psum_st.tile([128, GROUP], f32, tag="st",
                                    name=f"st_{h}_{g}_{kt}")
                for c in range(GROUP // 512):
                    nc.tensor.matmul(
                        st_t[:, c * 512:(c + 1) * 512],
                        lhsT=kt_ts[h][:, kt * 128:(kt + 1) * 128],
                        rhs=qt_ts[h][:, q0 + c * 512:q0 + (c + 1) * 512],
                        start=True, stop=True,
                    )
                # exp
                if kt in DVE_SET:
                    w_t = wk_pool.tile([128, GROUP], f32, tag="wk")
                    nc.vector._custom_dve(
                        op_expbits, out=w_t, in0=st_t,
                        s0=C0_MAGIC, s1=C1_QUAD, imm2=C2_QUAD)
                    pt_i = pti_pool.tile([128, GROUP], i16, tag="pti")
                    nc.gpsimd.tensor_scalar_add(pt_i, w_t, K_BITS)
                    pt_b = pt_i.bitcast(mybir.dt.bfloat16)
                    lag = LAG_DVE
                else:
                    pt_bt = ptb_pool.tile([128, GROUP], mybir.dt.bfloat16,
                                          tag="ptb")
                    nc.scalar.activation(
                        out=pt_bt, in_=st_t,
                        func=mybir.ActivationFunctionType.Exp,
                        bias=bias_t[:], scale=ACT_SCALE)
                    pt_b = pt_bt
                    lag = LAG_ACT
                pending.append((n + lag, h, g, kt, pt_b))
                while pending and pending[0][0] <= n:
                    _, fh, fg, fkt, fpt = pending.pop(0)
                    emit_pv(fh, fg, fkt, fpt)

            while pending:
                _, fh, fg, fkt, fpt = pending.pop(0)
                emit_pv(fh, fg, fkt, fpt)

    nc.compile()
    return nc


def _get_compiled():
    global _compiled
    if _compiled is None:
        _compiled = _build()
    return _compiled


def _to_bf16(x):
    b = np.ascontiguousarray(x, np.float32).view(np.uint32)
    r = ((b >> 16) + ((b >> 15) & 1)).astype(np.uint16)
    return r


def kernel(query: np.ndarray, key: np.ndarray, value: np.ndarray) -> np.ndarray:
    import ml_dtypes
    from concourse.bass_utils import run_bass_kernel_spmd

    nc = _get_compiled()

    q = np.asarray(query, dtype=np.float32).reshape(H, S, D)
    k = np.asarray(key, dtype=np.float32).reshape(H, S, D)
    v = np.asarray(value, dtype=np.float32).reshape(H, S, D)

    ksc = np.float32(128.0 * LOG2E)
    in_maps = []
    for c in range(N_CORES):
        hs = slice(c * HPC, (c + 1) * HPC)
        qh = q[hs].transpose(0, 2, 1)                       # [HPC, 64, S]
        kh = k[hs].transpose(0, 2, 1) * ksc                 # [HPC, 64, S]
        qT_host = np.concatenate(
            [qh, np.full((HPC, 1, S), SROW, np.float32)], axis=1)
        kT_host = np.concatenate(
            [kh, np.ones((HPC, 1, S), np.float32)], axis=1)
        v_host = np.concatenate(
            [v[hs], np.ones((HPC, S, 1), np.float32)], axis=-1)
        in_maps.append({
            "qT": np.ascontiguousarray(qT_host),
            "kT": np.ascontiguousarray(kT_host),
            "v": _to_bf16(v_host).view(ml_dtypes.bfloat16),
        })

    res = run_bass_kernel_spmd(nc, in_maps, list(range(N_CORES)))

    outp = np.empty((B, H, S, D), dtype=np.float32)
    for c in range(N_CORES):
        for hh in range(HPC):
            outp[0, c * HPC + hh] = res.results[c]["out"][hh]
    return outp


# revision 11
# speedup vs baseline: 1.1389x; 1.0626x over previous
"""Dense dot-product attention (B=1, H=16, S=4096, D=64, fp32) on 8 trn2 cores.

Head-parallel: core c computes heads [2c, 2c+1] fully on-device, no comms.

Per-head algorithm (S^T score layout, [q,d] output layout):
  scores: S^T[k, q] PSUM tile per (k-tile, q-group) via f32r matmuls with a
    65-row contraction: rows 0:64 contract d (K pre-scaled by 128*log2e),
    row 64 is a bias row adding SROW = 128*(127 - SH) - 64, so the PSUM
    value is v = 128*(x + 127) - 64 with x = score*log2e - SH.
  exp -> P (bf16) via two engine paths, split per k-tile:
    ACT path: P = Exp(v * ln2/128 + bias), full precision, bf16 out.
    DVE+Pool path ("expbits"): DVE custom op computes
        w = (C1*_t + C2)*_t + v,  _t = v - 128*round(v/128)  (magic round)
      then Pool adds K and converts to int16; the int16 IS the bf16 bit
      pattern of 2^x (exponent from the 128-quantized part, mantissa from
      the quadratic 2^frac fit; max weight err ~1%, rel out err ~3e-3).
  PV: out[q, d'] accumulates over 32 k-tiles via bf16 matmuls with
    lhsT = P^T[k, q-subtile] (ap=65/matmul), rhs = V'[k, 0:65] where V' has
    a ones column so out[q, 64] = sum of P (softmax denominator).
  normalize: per q-subtile reciprocal + per-partition scalar multiply;
    output written in natural [q, d] layout (no transposes).

PV matmuls are emitted LAG k-tiles behind QK so the exp chain latency never
stalls the in-order PE queue.
"""

import sys

if "/opt/trn_rl_repo" not in sys.path:
    sys.path.insert(0, "/opt/trn_rl_repo")

import numpy as np

B, H, S, D = 1, 16, 4096, 64
N_CORES = 8
HPC = H // N_CORES    # heads per core = 2

KT = S // 128         # 32 k-tiles per head
GROUP = 1024          # q columns per score group
NG = S // GROUP       # 4 groups per head
NJ = GROUP // 128     # 8 q-subtiles per group

LOG2E = 1.4426950408889634
LN2 = 0.6931471805599453
SH = 46.0                                   # shift in log2 units
SROW = 128.0 * (127.0 - SH) - 64.0          # bias-row constant (J = -64)
ACT_SCALE = LN2 / 128.0
ACT_BIAS = -(127.0 - 64.0 / 128.0) * LN2    # undoes 128*( . +127) - 64

# expbits constants: v + quad(_t) + K == bf16 bits of 2^x (see numcheck)
C0_MAGIC = 1.5 * 2**30
C1_QUAD = 2.459070897941e-03
C2_QUAD = -1.995185412854e-02
K_BITS = 53.044930589134

# k-tiles handled by the DVE+Pool expbits path (rest go to ACT).
# Evenly interleaved; none in the last tiles of a group so the group's PV
# tail flushes fast.
DVE_SET = frozenset({0, 2, 4, 6, 8, 10, 12, 14, 16, 18, 20, 23, 26})
LAG_ACT = 4
LAG_DVE = 9
DEBUG_DUMP = False

_compiled = None


def _register_expbits_op():
    import concourse.dve_ops as dve_ops
    from concourse.dve_ops import DveOp, OPS, has_src1
    from concourse.dve_spec import Spec, Src0, C0, C1, C2, lower
    from concourse.dve_uop import DveOpSpec

    if "EXPBITS_ANT" in dve_ops._SUB_OPCODE_FOR_NAME:
        return {op.name: op for op in OPS}["EXPBITS_ANT"]

    from concourse.dve_spec import Zero, maxx

    f32 = np.float32

    def ref(in0, in1, s0, s1, imm2):
        x = in0.astype(np.float32)
        r = x + f32(s0)
        s = r - f32(s0)
        t = x - s
        return np.maximum((t * f32(s1) + f32(imm2)) * t + x, f32(0.0))

    _r = Src0 + C0
    _t = Src0 - (_r - C0)
    op = DveOp(
        "EXPBITS_ANT",
        # max(.., 0): deeply negative scores (x < -127) would otherwise go
        # negative in int16 and bitcast to huge negative bf16 weights.
        Spec(body=maxx((_t * C1 + C2) * _t + Src0, Zero), reference=ref),
        subdim=False,
        uops_sha={},
    )
    OPS.append(op)
    dve_ops.CUSTOM_DVE_SPECS[op.name] = op.spec
    dve_ops._SUB_OPCODE_FOR_NAME[op.name] = (
        dve_ops._CUSTOM_DVE_ROW_BASE + len(dve_ops._SUB_OPCODE_FOR_NAME))
    for ver in ("v3", "v4"):
        try:
            compiled = DveOpSpec(
                name=op.name,
                opcode=dve_ops._SUB_OPCODE_FOR_NAME[op.name],
                uops=lower(op.spec, ver=ver),
                rd1_en=has_src1(op.spec),
            )
            op.uops_sha[ver] = compiled.sha(ver)
        except Exception:
            pass
    return op


def _build():
    import concourse.bacc as bacc
    import concourse.mybir as mybir
    import concourse.tile as tile

    op_expbits = _register_expbits_op()

    f32 = mybir.dt.float32
    f32r = mybir.dt.float32r
    bf16 = mybir.dt.bfloat16
    i16 = mybir.dt.int16

    nc = bacc.Bacc("TRN2", target_bir_lowering=False, debug=False,
                   num_devices=N_CORES)

    qT = nc.dram_tensor("qT", [HPC, D + 1, S], f32r, kind="ExternalInput")
    kT = nc.dram_tensor("kT", [HPC, D + 1, S], f32r, kind="ExternalInput")
    v = nc.dram_tensor("v", [HPC, S, D + 1], bf16, kind="ExternalInput")
    out = nc.dram_tensor("out", [HPC, S, D], f32, kind="ExternalOutput")
    dbg = None
    if DEBUG_DUMP:
        dbg = nc.dram_tensor("dbg", [HPC, NG, 128, NJ, 128], f32,
                             kind="ExternalOutput")

    with tile.TileContext(nc) as tc:
        with (
            tc.tile_pool(name="qk", bufs=2) as qk_pool,
            tc.tile_pool(name="vp", bufs=2) as vp_pool,
            tc.tile_pool(name="ptb", bufs=8) as ptb_pool,
            tc.tile_pool(name="pti", bufs=8) as pti_pool,
            tc.tile_pool(name="wk", bufs=8) as wk_pool,
            tc.tile_pool(name="osb", bufs=2) as osb_pool,
            tc.tile_pool(name="rcp", bufs=16) as rcp_pool,
            tc.tile_pool(name="small", bufs=1) as small_pool,
            tc.tile_pool(name="psum_st", bufs=3, space="PSUM") as psum_st,
            tc.tile_pool(name="psum_oa", bufs=1, space="PSUM") as psum_oa,
            tc.tile_pool(name="psum_ob", bufs=1, space="PSUM") as psum_ob,
        ):
            bias_t = small_pool.tile([128, 1], f32, tag="bias")
            nc.gpsimd.memset(bias_t, ACT_BIAS)
            # dummy exp so the ACT table set loads during the input DMAs
            warm_t = small_pool.tile([128, 1], f32, tag="warm")
            nc.scalar.activation(out=warm_t, in_=bias_t,
                                 func=mybir.ActivationFunctionType.Exp,
                                 bias=bias_t[:], scale=1.0)

            # ---- input loads; first head's first tiles load in small
            # chunks so the first QK starts as early as possible, and the
            # big transfers stay off the critical DMA path. ----
            kt_ts, qt_ts, vp_ts = {}, {}, {}
            for h in range(HPC):
                kt_ts[h] = qk_pool.tile([D + 1, S], f32r, tag="kt",
                                        name=f"kt_{h}")
                qt_ts[h] = qk_pool.tile([D + 1, S], f32r, tag="qt",
                                        name=f"qt_{h}")
                vp_ts[h] = vp_pool.tile([128, KT, D + 1], bf16, tag="vp",
                                        name=f"vp_{h}")
            nc.sync.dma_start(out=kt_ts[0][:, 0:512], in_=kT[0][:, 0:512])
            nc.sync.dma_start(out=qt_ts[0][:, 0:512], in_=qT[0][:, 0:512])
            nc.sync.dma_start(out=kt_ts[0][:, 512:2048],
                              in_=kT[0][:, 512:2048])
            nc.sync.dma_start(out=qt_ts[0][:, 512:GROUP],
                              in_=qT[0][:, 512:GROUP])
            nc.sync.dma_start(out=kt_ts[0][:, 2048:S], in_=kT[0][:, 2048:S])
            nc.scalar.dma_start(
                out=vp_ts[0],
                in_=v[0].rearrange("(kt p) e -> p kt e", p=128))
            nc.sync.dma_start(out=qt_ts[0][:, GROUP:S], in_=qT[0][:, GROUP:S])
            nc.sync.dma_start(out=kt_ts[1], in_=kT[1])
            nc.sync.dma_start(out=qt_ts[1], in_=qT[1])
            nc.scalar.dma_start(
                out=vp_ts[1],
                in_=v[1].rearrange("(kt p) e -> p kt e", p=128))

            # ---- flat pipeline over (h, g, kt) ----
            steps = [(h, g, kt)
                     for h in range(HPC) for g in range(NG) for kt in range(KT)]
            pending = []          # (release_at_step, h, g, kt, pt_bf16_ap)
            group_state = {}      # (h, g) -> dict(oa, ob, started, flushed)

            def get_group(h, g):
                key = (h, g)
                if key not in group_state:
                    group_state[key] = {
                        "oa": psum_oa.tile([128, 4, 128], f32, tag="oa",
                                           name=f"oa_{h}_{g}"),
                        "ob": psum_ob.tile([128, 4, 128], f32, tag="ob",
                                           name=f"ob_{h}_{g}"),
                        "bank_started": [False, False],
                        "flushed": 0,
                    }
                return group_state[key]

            def emit_pv(h, g, kt, pt_b):
                gs = get_group(h, g)
                last = (kt == KT - 1)
                for j in range(NJ):
                    bank = 0 if j < 4 else 1
                    o_ps = gs["oa"] if j < 4 else gs["ob"]
                    # start=True clears has_written for the WHOLE bank, so
                    # it must be issued exactly once per bank; later writes
                    # to virgin elements overwrite, the rest accumulate.
                    nc.tensor.matmul(
                        o_ps[:, j % 4, 0:D + 1],
                        lhsT=pt_b[:, j * 128:(j + 1) * 128],
                        rhs=vp_ts[h][:, kt, :],
                        start=(not gs["bank_started"][bank]), stop=last,
                        skip_group_check=True,
                    )
                    gs["bank_started"][bank] = True
                gs["flushed"] += 1
                if gs["flushed"] == KT:
                    emit_norm(h, g)

            def emit_norm(h, g):
                gs = group_state[(h, g)]
                if DEBUG_DUMP:
                    dbg_sb = osb_pool.tile([128, NJ, 128], f32, tag="dsb",
                                           name=f"dsb_{h}_{g}")
                    for j in range(NJ):
                        o_ps = gs["oa"] if j < 4 else gs["ob"]
                        nc.vector.tensor_copy(dbg_sb[:, j, :],
                                              o_ps[:, j % 4, :])
                    nc.sync.dma_start(out=dbg[h][g], in_=dbg_sb)
                out_sb = osb_pool.tile([128, NJ, D], f32, tag="osb",
                                       name=f"osb_{h}_{g}")
                for j in range(NJ):
                    o_ps = gs["oa"] if j < 4 else gs["ob"]
                    rcp_t = rcp_pool.tile([128, 1], f32, tag="rcp",
                                          name=f"rcp_{h}_{g}_{j}")
                    nc.vector.reciprocal(out=rcp_t,
                                         in_=o_ps[:, j % 4, D:D + 1])
                    nc.vector.tensor_scalar_mul(
                        out_sb[:, j, :], o_ps[:, j % 4, 0:D], rcp_t)
                nc.sync.dma_start(
                    out=out[h][g * GROUP:(g + 1) * GROUP, :].rearrange(
                        "(j p) d -> p j d", p=128),
                    in_=out_sb,
                )
                del group_state[(h, g)]

            for n, (h, g, kt) in enumerate(steps):
                q0 = g * GROUP
                # QK: 2 matmuls of 512 columns, 65-row contraction
                st_t = psum_st.tile([128, GROUP], f32, tag="st",
                                    name=f"st_{h}_{g}_{kt}")
                for c in range(GROUP // 512):
                    nc.tensor.matmul(
                        st_t[:, c * 512:(c + 1) * 512],
                        lhsT=kt_ts[h][:, kt * 128:(kt + 1) * 128],
                        rhs=qt_ts[h][:, q0 + c * 512:q0 + (c + 1) * 512],
                        start=True, stop=True,
                    )
                # exp
                if kt in DVE_SET:
                    w_t = wk_pool.tile([128, GROUP], f32, tag="wk")
                    nc.vector._custom_dve(
                        op_expbits, out=w_t, in0=st_t,
                        s0=C0_MAGIC, s1=C1_QUAD, imm2=C2_QUAD)
                    pt_i = pti_pool.tile([128, GROUP], i16, tag="pti")
                    nc.gpsimd.tensor_scalar_add(pt_i, w_t, K_BITS)
                    pt_b = pt_i.bitcast(mybir.dt.bfloat16)
                    lag = LAG_DVE
                else:
                    pt_bt = ptb_pool.tile([128, GROUP], mybir.dt.bfloat16,
                                          tag="ptb")
                    nc.scalar.activation(
                        out=pt_bt, in_=st_t,
                        func=mybir.ActivationFunctionType.Exp,
                        bias=bias_t[:], scale=ACT_SCALE)
                    pt_b = pt_bt
                    lag = LAG_ACT
                if h == HPC - 1 and g == NG - 1 and kt >= KT - 6:
                    lag = 1  # shorten the kernel tail
                pending.append((n + lag, h, g, kt, pt_b))
                while pending and pending[0][0] <= n:
                    _, fh, fg, fkt, fpt = pending.pop(0)
                    emit_pv(fh, fg, fkt, fpt)

            while pending:
                _, fh, fg, fkt, fpt = pending.pop(0)
                emit_pv(fh, fg, fkt, fpt)

    nc.compile()
    return nc


def _get_compiled():
    global _compiled
    if _compiled is None:
        _compiled = _build()
    return _compiled


def _to_bf16(x):
    b = np.ascontiguousarray(x, np.float32).view(np.uint32)
    r = ((b >> 16) + ((b >> 15) & 1)).astype(np.uint16)
    return r


def kernel(query: np.ndarray, key: np.ndarray, value: np.ndarray) -> np.ndarray:
    import ml_dtypes
    from concourse.bass_utils import run_bass_kernel_spmd

    nc = _get_compiled()

    q = np.asarray(query, dtype=np.float32).reshape(H, S, D)
    k = np.asarray(key, dtype=np.float32).reshape(H, S, D)
    v = np.asarray(value, dtype=np.float32).reshape(H, S, D)

    ksc = np.float32(128.0 * LOG2E)
    in_maps = []
    for c in range(N_CORES):
        hs = slice(c * HPC, (c + 1) * HPC)
        qh = q[hs].transpose(0, 2, 1)                       # [HPC, 64, S]
        kh = k[hs].transpose(0, 2, 1) * ksc                 # [HPC, 64, S]
        qT_host = np.concatenate(
            [qh, np.full((HPC, 1, S), SROW, np.float32)], axis=1)
        kT_host = np.concatenate(
            [kh, np.ones((HPC, 1, S), np.float32)], axis=1)
        v_host = np.concatenate(
            [v[hs], np.ones((HPC, S, 1), np.float32)], axis=-1)
        in_maps.append({
            "qT": np.ascontiguousarray(qT_host),
            "kT": np.ascontiguousarray(kT_host),
            "v": _to_bf16(v_host).view(ml_dtypes.bfloat16),
        })

    res = run_bass_kernel_spmd(nc, in_maps, list(range(N_CORES)))

    outp = np.empty((B, H, S, D), dtype=np.float32)
    for c in range(N_CORES):
        for hh in range(HPC):
            outp[0, c * HPC + hh] = res.results[c]["out"][hh]
    return outp
